# revision 10
# baseline (speedup 1.0000x reference)
"""Trainium2 Bass kernel for top-2 MoE (nn_MoE_2113123910117).

Strategy (expert-parallel, per sharding hint):
  - Host: router logits -> softmax -> top-2 -> normalized combine weights;
    dispatch tokens to 8 expert shards (one expert per NeuronCore) with a
    fixed per-expert device capacity of 2048 tokens (the perfect-balance
    share). The few tokens past capacity (capacity-overflow spill) are
    computed on host in fp32 and added during unshard.
  - Device (per core): SwiGLU expert FFN over its gathered tokens,
    y = diag(scale) @ ((silu(x Wg^T) * (x Wu^T)) Wd^T), fp16 matmul
    operands with fp32 PSUM accumulation. Gate/up weights are loaded in
    16 f-sliced tiles in compute order so the first matmul only waits for
    x-tile0 + the first 256KB weight slice (~5us) instead of the full Wg.
  - Host: scatter-add per-expert outputs back into the [B,T,D] output.

Self-contained: hardcodes all shapes from the problem spec.
"""

import os
import numpy as np

# recover automatically if a prior run left the NeuronCores wedged
os.environ.setdefault("NEURON_RT_RESET_CORES", "1")

D = 1024
FF = 2048
E = 8
TOPK = 2
NCORES = 8
ND = D // 128    # 8 contraction chunks
NF = FF // 128   # 16 ff chunks
TT = 512         # token tile (moving-operand N per matmul)
CAP = 2048       # device tokens per expert; overflow spills to host

# matmul operand dtype on device ("float16", "bfloat16")
MM_DTYPE = os.environ.get("MOE_MM_DTYPE", "float16")

# test-only knobs / results (harness never touches these)
LAST_RESULTS = None
_NC_CACHE = {}


def split_multi_waits(nc, mybir_mod):
    """This walrus build rejects any instruction carrying more than one
    sync wait ("Too many sync wait commands"). Hoist extra waits onto
    single-wait NOPs inserted just before the instruction on the same
    engine — semantically identical since engines execute in order."""
    n_split = 0
    for f in nc.m.functions:
        for blk in f.blocks:
            insts = blk.instructions
            newl = []
            changed = False
            for inst in insts:
                si = inst.sync_info
                if si is not None and len(si.on_wait) > 1:
                    waits = list(si.on_wait)
                    del si.on_wait[1:]
                    for j, w in enumerate(waits[1:]):
                        nop = mybir_mod.InstNoOp(
                            name=f"{inst.name}_w{j}",
                            engine=inst.engine,
                            ins=[],
                            outs=[],
                        )
                        nop.sync_info = mybir_mod.SyncInfo(on_wait=[w], on_update=[])
                        newl.append(nop)
                        n_split += 1
                    changed = True
                newl.append(inst)
            if changed:
                insts[:] = newl
    return n_split


def build_nc(cap=CAP, repeat=1):
    """Build the per-core Bass program: SwiGLU FFN for one expert over
    `cap` tokens. Same NEFF on all 8 cores (SPMD).

    repeat>1 wraps the whole body (including weight loads) in a hardware
    loop — used only for benchmarking (dispatch overhead amortization)."""
    import contextlib

    import concourse.bass as bass
    import concourse.mybir as mybir
    import concourse.tile as tile

    dt = mybir.dt
    f32 = dt.float32
    mmdt = getattr(dt, MM_DTYPE)
    AF = mybir.ActivationFunctionType
    ng = cap // 128
    nt = cap // TT
    assert cap % TT == 0

    nc = bass.Bass()
    # x^T arranged [128, d-chunk, token]; gate/up weights arranged
    # f-chunk-major so each [128, ND, 128] slice is one contiguous DMA
    xt = nc.dram_tensor("xt", [128, ND, cap], mmdt, kind="ExternalInput")
    wg = nc.dram_tensor("wg", [NF * 128, ND, 128], mmdt, kind="ExternalInput")
    wu = nc.dram_tensor("wu", [NF * 128, ND, 128], mmdt, kind="ExternalInput")
    wd = nc.dram_tensor("wd", [FF, D], mmdt, kind="ExternalInput")
    sc = nc.dram_tensor("sc", [128, ng], f32, kind="ExternalInput")
    y = nc.dram_tensor("y", [cap, D], f32, kind="ExternalOutput")

    with tile.TileContext(nc) as tc:
        with (
            tc.tile_pool(name="wpool", bufs=1) as wpool,
            tc.tile_pool(name="xpool", bufs=2) as xpool,
            tc.tile_pool(name="hpool", bufs=2) as hpool,
            tc.tile_pool(name="gpool", bufs=3) as gpool,
            tc.tile_pool(name="ypool", bufs=4) as ypool,
            tc.tile_pool(name="pg", bufs=2, space="PSUM") as pgpool,
            tc.tile_pool(name="pu", bufs=2, space="PSUM") as pupool,
            tc.tile_pool(name="po", bufs=4, space="PSUM") as popool,
        ):
            # PE warmup: a few matmuls on a zeroed scratch tile while the
            # first DMAs are in flight, so the HAM clock gate is already
            # at full rate when real matmuls start. Off the critical path
            # (PE would otherwise idle during the DMA lead-in). Outside the
            # benchmark repeat loop — only the first pass needs it.
            warm = wpool.tile([128, TT], mmdt, tag="warm")
            nc.gpsimd.memset(warm[:], 0)
            pwarm = pgpool.tile([128, TT], f32, tag="pg")
            for i in range(7):
                nc.tensor.matmul(pwarm[:], warm[:, 0:128], warm[:])
            rep_ctx = (
                tc.For_i(0, repeat, 1, hint_engines=(mybir.EngineType.PE,))
                if repeat > 1
                else contextlib.nullcontext()
            )
            rep_ctx.__enter__()
            # DMA issue order == compute-need order: x tile0, then
            # interleaved per-f gate/up weight slices, then down-proj
            # weights. Combine scales are first needed ~90us in.
            xt0 = xpool.tile([128, ND, TT], mmdt, tag="xt")
            nc.sync.dma_start(xt0[:], xt[:, :, 0:TT])
            wg_sb = []
            wu_sb = []
            s_sb = None
            for f in range(NF):
                tg = wpool.tile([128, ND, 128], mmdt, tag=f"wg{f}")
                nc.sync.dma_start(tg[:], wg[f * 128 : (f + 1) * 128, :, :])
                wg_sb.append(tg)
                tu = wpool.tile([128, ND, 128], mmdt, tag=f"wu{f}")
                nc.sync.dma_start(tu[:], wu[f * 128 : (f + 1) * 128, :, :])
                wu_sb.append(tu)
                if f == 0:
                    s_sb = wpool.tile([128, ng], f32, tag="s")
                    nc.sync.dma_start(s_sb[:], sc[:])
            wd_sb = []
            for f in range(NF):
                t = wpool.tile([128, D], mmdt, tag=f"wd{f}")
                nc.sync.dma_start(t[:], wd[f * 128 : (f + 1) * 128, :])
                wd_sb.append(t)

            for it in range(nt):
                off = it * TT
                if it == 0:
                    xt_t = xt0
                else:
                    xt_t = xpool.tile([128, ND, TT], mmdt, tag="xt")
                    nc.sync.dma_start(xt_t[:], xt[:, :, off : off + TT])
                # gate/up + SwiGLU -> h^T [f, tokens]
                ht_t = []
                for f in range(NF):
                    pg = pgpool.tile([128, TT], f32, tag="pg")
                    pu = pupool.tile([128, TT], f32, tag="pu")
                    for j in range(ND):
                        nc.tensor.matmul(
                            pg[:],
                            wg_sb[f][:, j, :],
                            xt_t[:, j, :],
                            start=(j == 0),
                            stop=(j == ND - 1),
                        )
                    for j in range(ND):
                        nc.tensor.matmul(
                            pu[:],
                            wu_sb[f][:, j, :],
                            xt_t[:, j, :],
                            start=(j == 0),
                            stop=(j == ND - 1),
                        )
                    sg = gpool.tile([128, TT], mmdt, tag="sg")
                    nc.scalar.activation(sg[:], pg[:], AF.Silu)
                    ht = hpool.tile([128, TT], mmdt, tag=f"ht{f}")
                    nc.vector.tensor_mul(ht[:], sg[:], pu[:])
                    ht_t.append(ht)
                # down projection, scaled by combine weight per token
                for k in range(TT // 128):
                    g = off // 128 + k
                    po_h = []
                    for dh in range(2):
                        po = popool.tile(
                            [128, 512], f32, tag="po", name=f"po_{it}_{k}_{dh}"
                        )
                        po_h.append(po)
                    for f in range(NF):
                        lhs = ht_t[f][:, k * 128 : (k + 1) * 128]
                        for dh in range(2):
                            nc.tensor.matmul(
                                po_h[dh][:],
                                lhs,
                                wd_sb[f][:, dh * 512 : (dh + 1) * 512],
                                start=(f == 0),
                                stop=(f == NF - 1),
                            )
                    # scale-by-combine-weight copies: dh=0 on ACT, dh=1 on
                    # DVE so the two run concurrently (shortens the kernel
                    # tail and halves ACT load)
                    for dh in range(2):
                        yt = ypool.tile([128, 512], f32, tag="yt")
                        if dh == 0:
                            nc.scalar.activation(
                                yt[:], po_h[dh][:], AF.Copy, scale=s_sb[:, g : g + 1]
                            )
                        else:
                            nc.vector.tensor_scalar_mul(
                                yt[:], po_h[dh][:], s_sb[:, g : g + 1]
                            )
                        # dh=1 stores go out on the Activation hwdge queue so
                        # the two stores of a group use parallel DGE queues
                        dge = nc.sync if dh == 0 else nc.scalar
                        dge.dma_start(
                            y[
                                off + k * 128 : off + (k + 1) * 128,
                                dh * 512 : (dh + 1) * 512,
                            ],
                            yt[:],
                        )
            rep_ctx.__exit__(None, None, None)
    split_multi_waits(nc, mybir)
    return nc


def _get_nc(cap=CAP):
    key = (cap, MM_DTYPE)
    if key not in _NC_CACHE:
        _NC_CACHE[key] = build_nc(cap)
    return _NC_CACHE[key]


def _route(xf, Wr):
    """fp32 softmax + top-2 + normalized combine weights, matching the
    jax reference (ties broken toward lower expert index)."""
    logits = xf @ Wr.astype(np.float32).T
    m = logits.max(-1, keepdims=True)
    ex = np.exp(logits - m)
    p = ex / ex.sum(-1, keepdims=True)
    top2 = np.argsort(-p, axis=-1, kind="stable")[:, :TOPK]
    n = xf.shape[0]
    p1 = p[np.arange(n), top2[:, 0]]
    p2 = p[np.arange(n), top2[:, 1]]
    denom = (p1 + p2) + np.float32(1e-8)
    return top2, p1 / denom, p2 / denom


def _prep_maps(inputs, cap=CAP):
    """Route + build per-core input maps. Returns
    (in_maps, dev_idxs, overflow, xf) where overflow is a list of
    (expert, token_idx_array, scale_array) for tokens past capacity."""
    x = np.asarray(inputs["x"])
    Wr = np.asarray(inputs["Wr"])
    Wg = np.asarray(inputs["Wg"])
    Wu = np.asarray(inputs["Wu"])
    Wd = np.asarray(inputs["Wd"])
    xf = x.reshape(-1, D).astype(np.float32, copy=False)

    top2, s1, s2 = _route(xf, Wr)

    mmnp = np.dtype(np.float16 if MM_DTYPE == "float16" else np.float32)
    if MM_DTYPE == "bfloat16":
        import ml_dtypes

        mmnp = np.dtype(ml_dtypes.bfloat16)
    xf_mm = xf.astype(mmnp)

    in_maps = []
    dev_idxs = []
    overflow = []
    for e in range(E):
        idx = np.nonzero((top2[:, 0] == e) | (top2[:, 1] == e))[0]
        sce = np.where(top2[idx, 0] == e, s1[idx], s2[idx]).astype(np.float32)
        n_dev = min(len(idx), cap)
        didx = idx[:n_dev]
        dev_idxs.append(didx)
        if n_dev < len(idx):
            overflow.append((e, idx[n_dev:], sce[n_dev:]))
        xt3 = np.zeros((128, ND, cap), dtype=mmnp)
        xt3[:, :, :n_dev] = xf_mm[didx].T.reshape(ND, 128, n_dev).transpose(1, 0, 2)
        scp = np.zeros(cap, dtype=np.float32)
        scp[:n_dev] = sce[:n_dev]
        wgT = Wg[e].T.reshape(ND, 128, NF, 128)
        wg4 = np.ascontiguousarray(
            wgT.transpose(2, 1, 0, 3).reshape(NF * 128, ND, 128)
        ).astype(mmnp)
        wuT = Wu[e].T.reshape(ND, 128, NF, 128)
        wu4 = np.ascontiguousarray(
            wuT.transpose(2, 1, 0, 3).reshape(NF * 128, ND, 128)
        ).astype(mmnp)
        in_maps.append(
            {
                "xt": xt3,
                "wg": wg4,
                "wu": wu4,
                "wd": np.ascontiguousarray(Wd[e].T).astype(mmnp),
                "sc": np.ascontiguousarray(scp.reshape(cap // 128, 128).T),
            }
        )
    return in_maps, dev_idxs, overflow, xf


def kernel(**inputs):
    global LAST_RESULTS
    from concourse.bass_utils import run_bass_kernel_spmd

    x = np.asarray(inputs["x"])
    B, T, _ = x.shape
    in_maps, dev_idxs, overflow, xf = _prep_maps(inputs)
    n_tok = xf.shape[0]

    nc = _get_nc(CAP)
    res = run_bass_kernel_spmd(nc, in_maps, list(range(NCORES)))
    LAST_RESULTS = res

    out = np.zeros((n_tok, D), dtype=np.float32)
    for e in range(E):
        didx = dev_idxs[e]
        out[didx] += res.results[e]["y"][: len(didx)]
    # host-side capacity-overflow spill (fp32, exact)
    if overflow:
        Wg = np.asarray(inputs["Wg"], dtype=np.float32)
        Wu = np.asarray(inputs["Wu"], dtype=np.float32)
        Wd = np.asarray(inputs["Wd"], dtype=np.float32)
        for e, oidx, osc in overflow:
            xo = xf[oidx]
            g = xo @ Wg[e].T
            u = xo @ Wu[e].T
            h = (g / (1.0 + np.exp(-g))) * u
            out[oidx] += osc[:, None] * (h @ Wd[e].T)
    return out.reshape(B, T, D).astype(x.dtype, copy=False)


# revision 14
# speedup vs baseline: 1.0502x; 1.0502x over previous
"""Trainium2 Bass kernel for top-2 MoE (nn_MoE_2113123910117).

Strategy (expert-parallel, per sharding hint):
  - Host: router logits -> softmax -> top-2 -> normalized combine weights;
    dispatch tokens to 8 expert shards (one expert per NeuronCore) with a
    fixed per-expert device capacity of 2048 tokens (the perfect-balance
    share). The few tokens past capacity (capacity-overflow spill) are
    computed on host in fp32 and added during unshard.
  - Device (per core): SwiGLU expert FFN over its gathered tokens,
    y = diag(scale) @ ((silu(x Wg^T) * (x Wu^T)) Wd^T), fp16 matmul
    operands with fp32 PSUM accumulation. Gate/up weights are loaded in
    16 f-sliced tiles in compute order so the first matmul only waits for
    x-tile0 + the first 256KB weight slice (~5us) instead of the full Wg.
  - Host: scatter-add per-expert outputs back into the [B,T,D] output.

Self-contained: hardcodes all shapes from the problem spec.
"""

import os
import numpy as np

# recover automatically if a prior run left the NeuronCores wedged
os.environ.setdefault("NEURON_RT_RESET_CORES", "1")

D = 1024
FF = 2048
E = 8
TOPK = 2
NCORES = 8
ND = D // 128    # 8 contraction chunks
NF = FF // 128   # 16 ff chunks
TT = 512         # token tile (moving-operand N per matmul)
CAP = 2048       # device tokens per expert; overflow spills to host

# matmul operand dtype on device ("float16", "bfloat16")
MM_DTYPE = os.environ.get("MOE_MM_DTYPE", "float16")

# test-only knobs / results (harness never touches these)
LAST_RESULTS = None
_NC_CACHE = {}


def split_multi_waits(nc, mybir_mod):
    """This walrus build rejects any instruction carrying more than one
    sync wait ("Too many sync wait commands"). Hoist extra waits onto
    single-wait NOPs inserted just before the instruction on the same
    engine — semantically identical since engines execute in order."""
    n_split = 0
    for f in nc.m.functions:
        for blk in f.blocks:
            insts = blk.instructions
            newl = []
            changed = False
            for inst in insts:
                si = inst.sync_info
                if si is not None and len(si.on_wait) > 1:
                    waits = list(si.on_wait)
                    del si.on_wait[1:]
                    for j, w in enumerate(waits[1:]):
                        nop = mybir_mod.InstNoOp(
                            name=f"{inst.name}_w{j}",
                            engine=inst.engine,
                            ins=[],
                            outs=[],
                        )
                        nop.sync_info = mybir_mod.SyncInfo(on_wait=[w], on_update=[])
                        newl.append(nop)
                        n_split += 1
                    changed = True
                newl.append(inst)
            if changed:
                insts[:] = newl
    return n_split


def build_nc(cap=CAP, repeat=1):
    """Build the per-core Bass program: SwiGLU FFN for one expert over
    `cap` tokens. Same NEFF on all 8 cores (SPMD).

    repeat>1 wraps the whole body (including weight loads) in a hardware
    loop — used only for benchmarking (dispatch overhead amortization)."""
    import contextlib

    import concourse.bass as bass
    import concourse.mybir as mybir
    import concourse.tile as tile

    dt = mybir.dt
    f32 = dt.float32
    mmdt = getattr(dt, MM_DTYPE)
    AF = mybir.ActivationFunctionType
    ng = cap // 128
    assert cap % TT == 0
    # first 512-token tile split in two 256s: the first matmul then only
    # waits for a 512KB x slice + one 256KB weight slice
    tiles = [(0, 256), (256, 256)]
    off = 256 + 256
    while off < cap:
        tiles.append((off, TT))
        off += TT

    nc = bass.Bass()
    # x^T arranged [128, d-chunk, token]; gate/up weights arranged
    # f-chunk-major so each [128, ND, 128] slice is one contiguous DMA
    xt = nc.dram_tensor("xt", [128, ND, cap], mmdt, kind="ExternalInput")
    wg = nc.dram_tensor("wg", [NF * 128, ND, 128], mmdt, kind="ExternalInput")
    wu = nc.dram_tensor("wu", [NF * 128, ND, 128], mmdt, kind="ExternalInput")
    wd = nc.dram_tensor("wd", [FF, D], mmdt, kind="ExternalInput")
    sc = nc.dram_tensor("sc", [128, ng], f32, kind="ExternalInput")
    y = nc.dram_tensor("y", [cap, D], f32, kind="ExternalOutput")

    with tile.TileContext(nc) as tc:
        with (
            tc.tile_pool(name="wpool", bufs=1) as wpool,
            tc.tile_pool(name="xpool", bufs=2) as xpool,
            tc.tile_pool(name="hpool", bufs=2) as hpool,
            tc.tile_pool(name="gpool", bufs=3) as gpool,
            tc.tile_pool(name="ypool", bufs=4) as ypool,
            tc.tile_pool(name="pg", bufs=2, space="PSUM") as pgpool,
            tc.tile_pool(name="pu", bufs=2, space="PSUM") as pupool,
            tc.tile_pool(name="po", bufs=4, space="PSUM") as popool,
        ):
            # PE warmup: a few matmuls on a zeroed scratch tile while the
            # first DMAs are in flight, so the HAM clock gate is already
            # at full rate when real matmuls start. Off the critical path
            # (PE would otherwise idle during the DMA lead-in). Outside the
            # benchmark repeat loop — only the first pass needs it.
            warm = wpool.tile([128, TT], mmdt, tag="warm")
            nc.gpsimd.memset(warm[:], 0)
            pwarm = pgpool.tile([128, TT], f32, tag="pg")
            for i in range(7):
                nc.tensor.matmul(pwarm[:], warm[:, 0:128], warm[:])
            rep_ctx = (
                tc.For_i(0, repeat, 1, hint_engines=(mybir.EngineType.PE,))
                if repeat > 1
                else contextlib.nullcontext()
            )
            rep_ctx.__enter__()
            # DMA issue order == compute-need order: x tile0, then
            # interleaved per-f gate/up weight slices, then down-proj
            # weights. Combine scales are first needed ~90us in.
            off0, tt0 = tiles[0]
            xt0 = xpool.tile([128, ND, tt0], mmdt, tag=f"xt{tt0}")
            nc.sync.dma_start(xt0[:], xt[:, :, off0 : off0 + tt0])
            wg_sb = []
            wu_sb = []
            s_sb = None
            for f in range(NF):
                tg = wpool.tile([128, ND, 128], mmdt, tag=f"wg{f}")
                nc.sync.dma_start(tg[:], wg[f * 128 : (f + 1) * 128, :, :])
                wg_sb.append(tg)
                tu = wpool.tile([128, ND, 128], mmdt, tag=f"wu{f}")
                nc.sync.dma_start(tu[:], wu[f * 128 : (f + 1) * 128, :, :])
                wu_sb.append(tu)
                if f == 0:
                    s_sb = wpool.tile([128, ng], f32, tag="s")
                    nc.sync.dma_start(s_sb[:], sc[:])
            wd_sb = []
            for f in range(NF):
                t = wpool.tile([128, D], mmdt, tag=f"wd{f}")
                nc.sync.dma_start(t[:], wd[f * 128 : (f + 1) * 128, :])
                wd_sb.append(t)

            for it, (off, tt) in enumerate(tiles):
                if it == 0:
                    xt_t = xt0
                else:
                    xt_t = xpool.tile([128, ND, tt], mmdt, tag=f"xt{tt}")
                    nc.sync.dma_start(xt_t[:], xt[:, :, off : off + tt])
                # gate/up + SwiGLU -> h^T [f, tokens]
                ht_t = []
                for f in range(NF):
                    pg = pgpool.tile([128, tt], f32, tag="pg")
                    pu = pupool.tile([128, tt], f32, tag="pu")
                    for j in range(ND):
                        nc.tensor.matmul(
                            pg[:],
                            wg_sb[f][:, j, :],
                            xt_t[:, j, :],
                            start=(j == 0),
                            stop=(j == ND - 1),
                        )
                    for j in range(ND):
                        nc.tensor.matmul(
                            pu[:],
                            wu_sb[f][:, j, :],
                            xt_t[:, j, :],
                            start=(j == 0),
                            stop=(j == ND - 1),
                        )
                    sg = gpool.tile([128, tt], mmdt, tag="sg")
                    nc.scalar.activation(sg[:], pg[:], AF.Silu)
                    ht = hpool.tile([128, tt], mmdt, tag=f"ht{f}")
                    nc.vector.tensor_mul(ht[:], sg[:], pu[:])
                    ht_t.append(ht)
                # down projection, scaled by combine weight per token
                for k in range(tt // 128):
                    g = off // 128 + k
                    po_h = []
                    for dh in range(2):
                        po = popool.tile(
                            [128, 512], f32, tag="po", name=f"po_{it}_{k}_{dh}"
                        )
                        po_h.append(po)
                    for f in range(NF):
                        lhs = ht_t[f][:, k * 128 : (k + 1) * 128]
                        for dh in range(2):
                            nc.tensor.matmul(
                                po_h[dh][:],
                                lhs,
                                wd_sb[f][:, dh * 512 : (dh + 1) * 512],
                                start=(f == 0),
                                stop=(f == NF - 1),
                            )
                    # scale-by-combine-weight copies: dh=0 on ACT, dh=1 on
                    # DVE so the two run concurrently (shortens the kernel
                    # tail and halves ACT load)
                    for dh in range(2):
                        yt = ypool.tile([128, 512], f32, tag="yt")
                        if dh == 0:
                            nc.scalar.activation(
                                yt[:], po_h[dh][:], AF.Copy, scale=s_sb[:, g : g + 1]
                            )
                        else:
                            nc.vector.tensor_scalar_mul(
                                yt[:], po_h[dh][:], s_sb[:, g : g + 1]
                            )
                        # dh=1 stores go out on the Activation hwdge queue so
                        # the two stores of a group use parallel DGE queues
                        dge = nc.sync if dh == 0 else nc.scalar
                        dge.dma_start(
                            y[
                                off + k * 128 : off + (k + 1) * 128,
                                dh * 512 : (dh + 1) * 512,
                            ],
                            yt[:],
                        )
            rep_ctx.__exit__(None, None, None)
    split_multi_waits(nc, mybir)
    return nc


def _get_nc(cap=CAP):
    key = (cap, MM_DTYPE)
    if key not in _NC_CACHE:
        _NC_CACHE[key] = build_nc(cap)
    return _NC_CACHE[key]


def _route(xf, Wr):
    """fp32 softmax + top-2 + normalized combine weights, matching the
    jax reference (ties broken toward lower expert index)."""
    logits = xf @ Wr.astype(np.float32).T
    m = logits.max(-1, keepdims=True)
    ex = np.exp(logits - m)
    p = ex / ex.sum(-1, keepdims=True)
    top2 = np.argsort(-p, axis=-1, kind="stable")[:, :TOPK]
    n = xf.shape[0]
    p1 = p[np.arange(n), top2[:, 0]]
    p2 = p[np.arange(n), top2[:, 1]]
    denom = (p1 + p2) + np.float32(1e-8)
    return top2, p1 / denom, p2 / denom


def _prep_maps(inputs, cap=CAP):
    """Route + build per-core input maps. Returns
    (in_maps, dev_idxs, overflow, xf) where overflow is a list of
    (expert, token_idx_array, scale_array) for tokens past capacity."""
    x = np.asarray(inputs["x"])
    Wr = np.asarray(inputs["Wr"])
    Wg = np.asarray(inputs["Wg"])
    Wu = np.asarray(inputs["Wu"])
    Wd = np.asarray(inputs["Wd"])
    xf = x.reshape(-1, D).astype(np.float32, copy=False)

    top2, s1, s2 = _route(xf, Wr)

    mmnp = np.dtype(np.float16 if MM_DTYPE == "float16" else np.float32)
    if MM_DTYPE == "bfloat16":
        import ml_dtypes

        mmnp = np.dtype(ml_dtypes.bfloat16)
    xf_mm = xf.astype(mmnp)

    in_maps = []
    dev_idxs = []
    overflow = []
    for e in range(E):
        idx = np.nonzero((top2[:, 0] == e) | (top2[:, 1] == e))[0]
        sce = np.where(top2[idx, 0] == e, s1[idx], s2[idx]).astype(np.float32)
        n_dev = min(len(idx), cap)
        didx = idx[:n_dev]
        dev_idxs.append(didx)
        if n_dev < len(idx):
            overflow.append((e, idx[n_dev:], sce[n_dev:]))
        xt3 = np.zeros((128, ND, cap), dtype=mmnp)
        xt3[:, :, :n_dev] = xf_mm[didx].T.reshape(ND, 128, n_dev).transpose(1, 0, 2)
        scp = np.zeros(cap, dtype=np.float32)
        scp[:n_dev] = sce[:n_dev]
        wgT = Wg[e].T.reshape(ND, 128, NF, 128)
        wg4 = np.ascontiguousarray(
            wgT.transpose(2, 1, 0, 3).reshape(NF * 128, ND, 128)
        ).astype(mmnp)
        wuT = Wu[e].T.reshape(ND, 128, NF, 128)
        wu4 = np.ascontiguousarray(
            wuT.transpose(2, 1, 0, 3).reshape(NF * 128, ND, 128)
        ).astype(mmnp)
        in_maps.append(
            {
                "xt": xt3,
                "wg": wg4,
                "wu": wu4,
                "wd": np.ascontiguousarray(Wd[e].T).astype(mmnp),
                "sc": np.ascontiguousarray(scp.reshape(cap // 128, 128).T),
            }
        )
    return in_maps, dev_idxs, overflow, xf


def kernel(**inputs):
    global LAST_RESULTS
    from concourse.bass_utils import run_bass_kernel_spmd

    x = np.asarray(inputs["x"])
    B, T, _ = x.shape
    in_maps, dev_idxs, overflow, xf = _prep_maps(inputs)
    n_tok = xf.shape[0]

    nc = _get_nc(CAP)
    res = run_bass_kernel_spmd(nc, in_maps, list(range(NCORES)))
    LAST_RESULTS = res

    out = np.zeros((n_tok, D), dtype=np.float32)
    for e in range(E):
        didx = dev_idxs[e]
        out[didx] += res.results[e]["y"][: len(didx)]
    # host-side capacity-overflow spill (fp32, exact)
    if overflow:
        Wg = np.asarray(inputs["Wg"], dtype=np.float32)
        Wu = np.asarray(inputs["Wu"], dtype=np.float32)
        Wd = np.asarray(inputs["Wd"], dtype=np.float32)
        for e, oidx, osc in overflow:
            xo = xf[oidx]
            g = xo @ Wg[e].T
            u = xo @ Wu[e].T
            h = (g / (1.0 + np.exp(-g))) * u
            out[oidx] += osc[:, None] * (h @ Wd[e].T)
    return out.reshape(B, T, D).astype(x.dtype, copy=False)
